# revision 13
# baseline (speedup 1.0000x reference)
"""Viterbi decode (CRF) kernel for Trainium2.

Problem: feats [T=8192, L=512] emissions, transitions [L, L] (trans[i, j] =
score of i -> j). Returns (viterbi_score [], viterbi_path [T] int32),
matching the jax reference (lax.scan forward max-plus + backtrack).

Device strategy (8 NeuronCores, single NEFF, SPMD):

Phase 1 (all cores redundantly -- the scan is inherently serial):
  Layout A: source labels i on partitions (4 blocks of 128), dest labels j
  on the free dim. State s4p[p, k] = scores[128k + p] (per-partition
  scalars). Per step:
    - DVE: x = (trans_b1 + s_1); x = (trans_b0 + s_0) max x;
           x = (trans_b2 + s_2) max x; M = (trans_b3 + s_3) max x
      (tensor_scalar + 3x scalar_tensor_tensor, fused add+max tree)
    - PE: 4x transpose of M's 128-col blocks into one PSUM bank
    - DVE: one tensor_reduce(max) over [128, 4, 128] -> a4 [128, 4]
      (a4[p, b] = max_i(s[i] + trans[i, 128b + p]), pre-emission)
    - DVE: s4p = a4 + e4 (emission, partition-form)
    - off-chain: PE transpose s4p -> [4, 128] PSUM -> DMA as score row u
      to internal DRAM; a4 ring-buffered and flushed to internal DRAM.

Phase 2 (time-sharded across cores by partition_id): recompute m rows in
  layout B (dest j on partitions via transposed transitions) from stored
  score rows (K=1 fp32 matmul broadcast, bitwise-exact), take stored a4 as
  the exact max, and max_index -> first argmax index == jnp.argmax ties.

Host: trivial O(T) backtrack over the returned backpointer array.
"""

import os
import numpy as np

T_FULL = 8192
L = 512
P = 128
NB = 4          # label blocks of 128
CH = 32         # steps per hardware-loop chunk
NCORES = 8
NEG = -3.0e38

_cache = {}


def _wrap_birfix(nc):
    """Split multi-wait instructions into single-wait NoOp chains.

    This walrus build's CTRL encoder accepts only one semaphore wait per
    instruction; the Tile kernel-tail drain can carry several. Splitting is
    semantically identical (all waits still precede the instruction).
    Scoped to this Bass instance only.
    """
    import json as _json
    import types as _types
    import concourse.bass as _bass

    orig = _bass.Bass.to_json_bytes
    counter = [0]

    def to_json_bytes(self):
        bir = _json.loads(orig(self))
        changed = False
        for fn in bir.get("functions", []):
            for blk in fn.get("blocks", []):
                out = []
                for ins in blk.get("instructions", []):
                    si = ins.get("sync_info")
                    waits = (si or {}).get("on_wait") or []
                    if len(waits) > 1:
                        changed = True
                        for w in waits[:-1]:
                            counter[0] += 1
                            out.append({
                                "engine": ins["engine"],
                                "ins": [],
                                "name": f"IWSPLIT-{counter[0]}",
                                "opcode": "NoOp",
                                "outs": [],
                                "sync_info": {"on_update": [], "on_wait": [w]},
                            })
                        si["on_wait"] = waits[-1:]
                    out.append(ins)
                blk["instructions"] = out
        return _json.dumps(bir).encode() if changed else orig(self)

    nc.to_json_bytes = _types.MethodType(to_json_bytes, nc)
    return nc


def _build(T):
    import concourse.bass as bass
    import concourse.mybir as mybir
    from concourse.bass import ds
    from concourse.tile import TileContext
    from concourse.masks import make_identity

    f32 = mybir.dt.float32
    u16 = mybir.dt.uint16
    ALU = mybir.AluOpType

    steps = T - 1                 # scan steps u = 1..T-1
    nch1 = steps // CH            # full phase-1 chunks
    tail1 = steps - nch1 * CH     # python-unrolled tail steps
    pc_steps = T // NCORES        # phase-2 steps per core
    ncq = pc_steps // CH          # phase-2 chunks per core

    nc = _wrap_birfix(bass.Bass())

    transA_d = nc.dram_tensor("transA", [P, NB, L], f32, kind="ExternalInput")
    transT_d = nc.dram_tensor("transT", [P, NB, L], f32, kind="ExternalInput")
    featsT4_d = nc.dram_tensor("featsT4", [P, T, NB], f32, kind="ExternalInput")

    rows_d = nc.dram_tensor("rows_i", [NB, T, P], f32, kind="Internal")
    a4_d = nc.dram_tensor("a4_i", [P, T + CH, NB], f32, kind="Internal")

    frow_d = nc.dram_tensor("final_row", [1, L], f32, kind="ExternalOutput")
    bp_d = nc.dram_tensor("bp", [P, ncq, CH, NB], u16, kind="ExternalOutput")

    with TileContext(nc) as tc:
        with (
            tc.tile_pool(name="consts", bufs=1) as consts,
            tc.tile_pool(name="state", bufs=1) as statep,
        ):
            ident = consts.tile([P, P], f32, tag="ident")
            make_identity(nc, ident)
            ones_row = consts.tile([1, P], f32, tag="ones_row")
            nc.vector.memset(ones_row, 1.0)
            s4p = statep.tile([P, NB], f32, tag="s4p")

            # ---------------- Phase 1: serial forward scan ----------------
            with (
                tc.tile_pool(name="p1sbuf", bufs=3) as pool,
                tc.tile_pool(name="p1ring", bufs=2) as ringp,
                tc.tile_pool(name="p1psum", bufs=2, space="PSUM") as psum,
                tc.tile_pool(name="p1psrow", bufs=4, space="PSUM") as psrow,
            ):
                transA = consts.tile([P, NB, L], f32, tag="transA")
                nc.sync.dma_start(transA, transA_d[:, :, :])

                # init: s4p = e4(0) = feats[0] in partition form
                e40 = pool.tile([P, 1, NB], f32, tag="e4chunk")
                nc.sync.dma_start(e40, featsT4_d[:, 0:1, :])
                nc.vector.tensor_copy(s4p, e40[:, 0, :])
                # scores_rows[0] = feats[0]
                srow0 = psrow.tile([NB, P], f32, tag="srow")
                nc.tensor.transpose(srow0, s4p, ident)
                srow0_sb = pool.tile([NB, P], f32, tag="srow0sb")
                nc.scalar.copy(srow0_sb, srow0)
                nc.sync.dma_start(rows_d[:, 0:1, :], srow0_sb)

                def step(u_expr, e4_ap, a4_slot_ap, row_slot_ap):
                    """One scan step; u_expr only used via the passed APs."""
                    x = pool.tile([P, L], f32, tag="x")
                    nc.vector.tensor_scalar(
                        x, transA[:, 1, :], s4p[:, 1:2], None, op0=ALU.add
                    )
                    for b in (0, 2):
                        nc.vector.scalar_tensor_tensor(
                            out=x, in0=transA[:, b, :], scalar=s4p[:, b:b + 1],
                            in1=x, op0=ALU.add, op1=ALU.max,
                        )
                    M = pool.tile([P, L], f32, tag="M")
                    nc.vector.scalar_tensor_tensor(
                        out=M, in0=transA[:, 3, :], scalar=s4p[:, 3:4],
                        in1=x, op0=ALU.add, op1=ALU.max,
                    )
                    MT = psum.tile([P, NB, P], f32, tag="MT")
                    for b in range(NB):
                        nc.tensor.transpose(MT[:, b, :], M[:, b * P:(b + 1) * P], ident)
                    a4 = pool.tile([P, NB], f32, tag="a4")
                    nc.vector.tensor_reduce(a4, MT, axis=mybir.AxisListType.X, op=ALU.max)
                    # stash pre-emission maxes (exact) for phase 2
                    nc.scalar.copy(a4_slot_ap, a4)
                    # s4p_new = a4 + emission(u)
                    nc.vector.tensor_tensor(s4p, a4, e4_ap, op=ALU.add)
                    # row store: transpose s4p -> [4, 128] -> SBUF ring slot
                    srow = psrow.tile([NB, P], f32, tag="srow")
                    nc.tensor.transpose(srow, s4p, ident)
                    nc.scalar.copy(row_slot_ap, srow)

                with tc.For_i(0, nch1, 1) as ci:
                    base = ci * CH + 1   # first u of this chunk
                    e4c = pool.tile([P, CH, NB], f32, tag="e4chunk")
                    nc.sync.dma_start(e4c, featsT4_d[:, ds(base, CH), :])
                    a4buf = ringp.tile([P, CH, NB], f32, tag="a4buf")
                    rowbuf = ringp.tile([NB, CH, P], f32, tag="rowbuf")
                    for t in range(CH):
                        step(
                            None,
                            e4c[:, t, :],
                            a4buf[:, t, :],
                            rowbuf[:, t, :],
                        )
                    nc.sync.dma_start(a4_d[:, ds(base, CH), :], a4buf)
                    nc.sync.dma_start(rows_d[:, ds(base, CH), :], rowbuf)

                if tail1:
                    base = nch1 * CH + 1
                    e4c = pool.tile([P, CH, NB], f32, tag="e4chunk")
                    nc.sync.dma_start(e4c[:, 0:tail1, :], featsT4_d[:, base:base + tail1, :])
                    a4buf = ringp.tile([P, CH, NB], f32, tag="a4buf")
                    rowbuf = ringp.tile([NB, CH, P], f32, tag="rowbuf")
                    for t in range(tail1):
                        step(
                            None,
                            e4c[:, t, :],
                            a4buf[:, t, :],
                            rowbuf[:, t, :],
                        )
                    nc.sync.dma_start(
                        a4_d[:, base:base + tail1, :], a4buf[:, 0:tail1, :]
                    )
                    nc.sync.dma_start(
                        rows_d[:, base:base + tail1, :], rowbuf[:, 0:tail1, :]
                    )

            tc.strict_bb_all_engine_barrier()

            # ---------------- Phase 2: backpointers, time-sharded ----------------
            with (
                tc.tile_pool(name="p2sbuf", bufs=3) as pool,
                tc.tile_pool(name="p2m", bufs=3) as mpool,
                tc.tile_pool(name="p2psum", bufs=2, space="PSUM") as psum,
            ):
                transT = consts.tile([P, NB, L], f32, tag="transT")
                nc.sync.dma_start(transT, transT_d[:, :, :])

                # final scores row for the host (after the barrier so the
                # phase-1 row writes are complete)
                frow_sb = pool.tile([1, L], f32, tag="frowsb")
                nc.sync.dma_start(frow_sb, rows_d[:, T - 1:T, :])
                nc.sync.dma_start(frow_d[:, :], frow_sb)
                inmax = consts.tile([P, NB, 8], f32, tag="inmax")
                nc.vector.memset(inmax, 3.0e38)

                pid = nc.partition_id()

                with tc.For_i(0, ncq, 1) as q:
                    vbase = pid * pc_steps + q * CH   # v = u - 1 for 1st step
                    a4c = pool.tile([P, CH, NB], f32, tag="a4c")
                    nc.sync.dma_start(a4c, a4_d[:, ds(vbase + 1, CH), :])
                    rows_c = pool.tile([NB, CH, P], f32, tag="rows_c")
                    nc.sync.dma_start(rows_c, rows_d[:, ds(vbase, CH), :])
                    bpring = pool.tile([P, CH, NB], u16, tag="bpring")
                    for t in range(CH):
                        rowt = pool.tile([1, L], f32, tag="rowt")
                        nc.sync.dma_start(rowt, rows_c[:, t, :])
                        rep = psum.tile([P, L], f32, tag="rep")
                        nc.tensor.matmul(
                            rep, ones_row, rowt, start=True, stop=True
                        )
                        # exact max values into slot 0 of the 8-wide in_max rows
                        nc.vector.tensor_copy(inmax[:, :, 0:1], a4c[:, t:t + 1, :].rearrange("p a b -> p b a"))
                        mo = mpool.tile([P, NB, 8], u16, tag="midxo")
                        for k in range(NB):
                            m = mpool.tile([P, L], f32, tag="m")
                            nc.vector.tensor_tensor(m, transT[:, k, :], rep, op=ALU.add)
                            nc.vector.max_index(mo[:, k, :], inmax[:, k, :], m)
                        nc.vector.tensor_copy(
                            bpring[:, t, :], mo[:, :, 0:1].rearrange("p a b -> p (a b)")
                        )
                    nc.sync.dma_start(bp_d[:, ds(q, 1), :, :], bpring)

    return nc


def _prep_inputs(feats, transitions, T):
    transA = np.ascontiguousarray(
        transitions.reshape(NB, P, L).transpose(1, 0, 2)
    )                                                  # [p, k, j] = trans[128k+p, j]
    transT = np.ascontiguousarray(
        transitions.T.reshape(NB, P, L).transpose(1, 0, 2)
    )                                                  # [p, k, i] = trans[i, 128k+p]
    featsT4 = np.ascontiguousarray(
        feats.reshape(T, NB, P).transpose(2, 0, 1)
    )                                                  # [p, u, k] = feats[u, 128k+p]
    return {"transA": transA, "transT": transT, "featsT4": featsT4}


def _run_device(feats, transitions, T, trace=False):
    from concourse import bass_utils

    key = T
    if key not in _cache:
        _cache[key] = _build(T)
    nc = _cache[key]

    in_map = _prep_inputs(feats, transitions, T)
    res = bass_utils.run_bass_kernel_spmd(
        nc, [in_map] * NCORES, core_ids=list(range(NCORES)), trace=trace,
    )
    return res


def _assemble(res, T):
    pc_steps = T // NCORES
    final_row = np.asarray(res.results[0]["final_row"]).reshape(L)
    # bp per core: [P, ncq, CH, NB] uint16
    bps = [np.asarray(res.results[c]["bp"]) for c in range(NCORES)]

    last_tag = int(np.argmax(final_row))
    score = np.float32(final_row[last_tag])

    path = np.empty(T, dtype=np.int32)
    path[T - 1] = last_tag
    tag = last_tag
    for v in range(T - 2, -1, -1):
        c, r = divmod(v, pc_steps)
        q, t = divmod(r, CH)
        tag = int(bps[c][tag % P, q, t, tag // P])
        path[v] = tag
    return score, path


def kernel(feats, transitions):
    feats = np.ascontiguousarray(feats, dtype=np.float32)
    transitions = np.ascontiguousarray(transitions, dtype=np.float32)
    T = feats.shape[0]
    res = _run_device(feats, transitions, T)
    score, path = _assemble(res, T)
    return score, path


if __name__ == "__main__":
    rng = np.random.default_rng(0)
    T = int(os.environ.get("VITERBI_T", "1024"))
    feats = rng.standard_normal((T, L), dtype=np.float32)
    trans = rng.standard_normal((L, L), dtype=np.float32)
    score, path = kernel(feats, trans)
    print("score", score, "path head", path[:8])


# revision 16
# speedup vs baseline: 57.5590x; 57.5590x over previous
"""Viterbi decode (CRF) kernel for Trainium2.

Problem: feats [T=8192, L=512] emissions, transitions [L, L] (trans[i, j] =
score of i -> j). Returns (viterbi_score [], viterbi_path [T] int32),
matching the jax reference (lax.scan forward max-plus + backtrack).

Device strategy (8 NeuronCores, single NEFF, SPMD):

Phase 1 (all cores redundantly -- the scan is inherently serial):
  Layout A: source labels i on partitions (4 blocks of 128), dest labels j
  on the free dim. State s4p[p, k] = scores[128k + p] (per-partition
  scalars). Per step:
    - DVE: x = (trans_b1 + s_1); x = (trans_b0 + s_0) max x;
           x = (trans_b2 + s_2) max x; M = (trans_b3 + s_3) max x
      (tensor_scalar + 3x scalar_tensor_tensor, fused add+max tree)
    - PE: 4x transpose of M's 128-col blocks into one PSUM bank
    - DVE: one tensor_reduce(max) over [128, 4, 128] -> a4 [128, 4]
      (a4[p, b] = max_i(s[i] + trans[i, 128b + p]), pre-emission)
    - DVE: s4p = a4 + e4 (emission, partition-form)
    - off-chain: PE transpose s4p -> [4, 128] PSUM -> DMA as score row u
      to internal DRAM; a4 ring-buffered and flushed to internal DRAM.

Phase 2 (time-sharded across cores by partition_id): recompute m rows in
  layout B (dest j on partitions via transposed transitions) from stored
  score rows (K=1 fp32 matmul broadcast, bitwise-exact), take stored a4 as
  the exact max, and max_index -> first argmax index == jnp.argmax ties.

Host: trivial O(T) backtrack over the returned backpointer array.
"""

import os
import numpy as np

T_FULL = 8192
L = 512
P = 128
NB = 4          # label blocks of 128
CH = 32         # steps per hardware-loop chunk
NCORES = 8
NEG = -3.0e38

_cache = {}


def _wrap_birfix(nc):
    """Split multi-wait instructions into single-wait NoOp chains.

    This walrus build's CTRL encoder accepts only one semaphore wait per
    instruction; the Tile kernel-tail drain can carry several. Splitting is
    semantically identical (all waits still precede the instruction).
    Scoped to this Bass instance only.
    """
    import json as _json
    import types as _types
    import concourse.bass as _bass

    orig = _bass.Bass.to_json_bytes
    counter = [0]

    def to_json_bytes(self):
        bir = _json.loads(orig(self))
        changed = False
        for fn in bir.get("functions", []):
            for blk in fn.get("blocks", []):
                out = []
                for ins in blk.get("instructions", []):
                    si = ins.get("sync_info")
                    waits = (si or {}).get("on_wait") or []
                    if len(waits) > 1:
                        changed = True
                        for w in waits[:-1]:
                            counter[0] += 1
                            out.append({
                                "engine": ins["engine"],
                                "ins": [],
                                "name": f"IWSPLIT-{counter[0]}",
                                "opcode": "NoOp",
                                "outs": [],
                                "sync_info": {"on_update": [], "on_wait": [w]},
                            })
                        si["on_wait"] = waits[-1:]
                    out.append(ins)
                blk["instructions"] = out
        return _json.dumps(bir).encode() if changed else orig(self)

    nc.to_json_bytes = _types.MethodType(to_json_bytes, nc)
    return nc


def _build(T):
    import concourse.bass as bass
    import concourse.mybir as mybir
    from concourse.bass import ds
    from concourse.tile import TileContext
    from concourse.masks import make_identity

    f32 = mybir.dt.float32
    u16 = mybir.dt.uint16
    ALU = mybir.AluOpType

    steps = T - 1                 # scan steps u = 1..T-1
    nch1 = steps // CH            # full phase-1 chunks
    tail1 = steps - nch1 * CH     # python-unrolled tail steps
    pc_steps = T // NCORES        # phase-2 steps per core
    ncq = pc_steps // CH          # phase-2 chunks per core

    nc = _wrap_birfix(bass.Bass())

    transA_d = nc.dram_tensor("transA", [P, NB, L], f32, kind="ExternalInput")
    transT_d = nc.dram_tensor("transT", [P, NB, L], f32, kind="ExternalInput")
    featsT4_d = nc.dram_tensor("featsT4", [P, T, NB], f32, kind="ExternalInput")

    rows_d = nc.dram_tensor("rows_i", [NB, T, P], f32, kind="Internal")
    a4_d = nc.dram_tensor("a4_i", [P, T + CH, NB], f32, kind="Internal")

    frow_d = nc.dram_tensor("final_row", [1, L], f32, kind="ExternalOutput")
    bp_d = nc.dram_tensor("bp", [P, ncq, CH, NB], u16, kind="ExternalOutput")

    with TileContext(nc) as tc:
        with (
            tc.tile_pool(name="consts", bufs=1) as consts,
            tc.tile_pool(name="state", bufs=1) as statep,
        ):
            ident = consts.tile([P, P], f32, tag="ident")
            make_identity(nc, ident)
            ones_row = consts.tile([1, P], f32, tag="ones_row")
            nc.vector.memset(ones_row, 1.0)
            s4p = statep.tile([P, NB], f32, tag="s4p")

            # ---------------- Phase 1: serial forward scan ----------------
            with (
                tc.tile_pool(name="p1sbuf", bufs=3) as pool,
                tc.tile_pool(name="p1ring", bufs=2) as ringp,
                tc.tile_pool(name="p1psum", bufs=2, space="PSUM") as psum,
                tc.tile_pool(name="p1psrow", bufs=4, space="PSUM") as psrow,
            ):
                transA = consts.tile([P, NB, L], f32, tag="transA")
                nc.sync.dma_start(transA, transA_d[:, :, :])

                # init: s4p = e4(0) = feats[0] in partition form
                e40 = pool.tile([P, 1, NB], f32, tag="e4chunk")
                nc.sync.dma_start(e40, featsT4_d[:, 0:1, :])
                nc.vector.tensor_copy(s4p, e40[:, 0, :])
                # scores_rows[0] = feats[0]
                srow0 = psrow.tile([NB, P], f32, tag="srow")
                nc.tensor.transpose(srow0, s4p, ident)
                srow0_sb = pool.tile([NB, P], f32, tag="srow0sb")
                nc.scalar.copy(srow0_sb, srow0)
                nc.sync.dma_start(rows_d[:, 0:1, :], srow0_sb)

                def step(u_expr, e4_ap, a4_slot_ap, row_slot_ap):
                    """One scan step; u_expr only used via the passed APs."""
                    x = pool.tile([P, L], f32, tag="x")
                    nc.vector.tensor_scalar(
                        x, transA[:, 1, :], s4p[:, 1:2], None, op0=ALU.add
                    )
                    for b in (0, 2):
                        nc.vector.scalar_tensor_tensor(
                            out=x, in0=transA[:, b, :], scalar=s4p[:, b:b + 1],
                            in1=x, op0=ALU.add, op1=ALU.max,
                        )
                    M = pool.tile([P, L], f32, tag="M")
                    MT = psum.tile([P, NB, P], f32, tag="MT")
                    for b in range(NB):
                        cs = slice(b * P, (b + 1) * P)
                        nc.vector.scalar_tensor_tensor(
                            out=M[:, cs], in0=transA[:, 3, cs], scalar=s4p[:, 3:4],
                            in1=x[:, cs], op0=ALU.add, op1=ALU.max,
                        )
                        nc.tensor.transpose(MT[:, b, :], M[:, cs], ident)
                    a4 = pool.tile([P, NB], f32, tag="a4")
                    nc.vector.tensor_reduce(a4, MT, axis=mybir.AxisListType.X, op=ALU.max)
                    # stash pre-emission maxes (exact) for phase 2
                    nc.scalar.copy(a4_slot_ap, a4)
                    # s4p_new = a4 + emission(u)
                    nc.vector.tensor_tensor(s4p, a4, e4_ap, op=ALU.add)
                    # row store: transpose s4p -> [4, 128] -> SBUF ring slot
                    srow = psrow.tile([NB, P], f32, tag="srow")
                    nc.tensor.transpose(srow, s4p, ident)
                    nc.scalar.copy(row_slot_ap, srow)

                with tc.For_i(0, nch1, 1) as ci:
                    base = ci * CH + 1   # first u of this chunk
                    e4c = pool.tile([P, CH, NB], f32, tag="e4chunk")
                    nc.sync.dma_start(e4c, featsT4_d[:, ds(base, CH), :])
                    a4buf = ringp.tile([P, CH, NB], f32, tag="a4buf")
                    rowbuf = ringp.tile([NB, CH, P], f32, tag="rowbuf")
                    for t in range(CH):
                        step(
                            None,
                            e4c[:, t, :],
                            a4buf[:, t, :],
                            rowbuf[:, t, :],
                        )
                    nc.sync.dma_start(a4_d[:, ds(base, CH), :], a4buf)
                    nc.sync.dma_start(rows_d[:, ds(base, CH), :], rowbuf)

                if tail1:
                    base = nch1 * CH + 1
                    e4c = pool.tile([P, CH, NB], f32, tag="e4chunk")
                    nc.sync.dma_start(e4c[:, 0:tail1, :], featsT4_d[:, base:base + tail1, :])
                    a4buf = ringp.tile([P, CH, NB], f32, tag="a4buf")
                    rowbuf = ringp.tile([NB, CH, P], f32, tag="rowbuf")
                    for t in range(tail1):
                        step(
                            None,
                            e4c[:, t, :],
                            a4buf[:, t, :],
                            rowbuf[:, t, :],
                        )
                    nc.sync.dma_start(
                        a4_d[:, base:base + tail1, :], a4buf[:, 0:tail1, :]
                    )
                    nc.sync.dma_start(
                        rows_d[:, base:base + tail1, :], rowbuf[:, 0:tail1, :]
                    )

            tc.strict_bb_all_engine_barrier()

            # ---------------- Phase 2: backpointers, time-sharded ----------------
            with (
                tc.tile_pool(name="p2sbuf", bufs=3) as pool,
                tc.tile_pool(name="p2m", bufs=3) as mpool,
                tc.tile_pool(name="p2psum", bufs=2, space="PSUM") as psum,
            ):
                transT = consts.tile([P, NB, L], f32, tag="transT")
                nc.sync.dma_start(transT, transT_d[:, :, :])

                # final scores row for the host (after the barrier so the
                # phase-1 row writes are complete)
                frow_sb = pool.tile([1, L], f32, tag="frowsb")
                nc.sync.dma_start(frow_sb, rows_d[:, T - 1:T, :])
                nc.sync.dma_start(frow_d[:, :], frow_sb)
                inmax = consts.tile([P, NB, 8], f32, tag="inmax")
                nc.vector.memset(inmax, 3.0e38)

                pid = nc.partition_id()

                with tc.For_i(0, ncq, 1) as q:
                    vbase = pid * pc_steps + q * CH   # v = u - 1 for 1st step
                    a4c = pool.tile([P, CH, NB], f32, tag="a4c")
                    nc.sync.dma_start(a4c, a4_d[:, ds(vbase + 1, CH), :])
                    rows_c = pool.tile([NB, CH, P], f32, tag="rows_c")
                    nc.sync.dma_start(rows_c, rows_d[:, ds(vbase, CH), :])
                    bpring = pool.tile([P, CH, NB], u16, tag="bpring")
                    for t in range(CH):
                        rowt = pool.tile([1, L], f32, tag="rowt")
                        nc.sync.dma_start(rowt, rows_c[:, t, :])
                        rep = psum.tile([P, L], f32, tag="rep")
                        nc.tensor.matmul(
                            rep, ones_row, rowt, start=True, stop=True
                        )
                        rep_sb = mpool.tile([P, L], f32, tag="rep_sb")
                        nc.scalar.copy(rep_sb, rep)
                        # exact max values into slot 0 of the 8-wide in_max rows
                        nc.vector.tensor_copy(inmax[:, :, 0:1], a4c[:, t:t + 1, :].rearrange("p a b -> p b a"))
                        mo = mpool.tile([P, NB, 8], u16, tag="midxo")
                        ms = []
                        for k in range(NB):
                            m = mpool.tile([P, L], f32, tag="m")
                            eng = nc.gpsimd if k in (0, 1) else nc.vector
                            eng.tensor_tensor(m, transT[:, k, :], rep_sb, op=ALU.add)
                            ms.append(m)
                        for k in range(NB):
                            nc.vector.max_index(mo[:, k, :], inmax[:, k, :], ms[k])
                        nc.vector.tensor_copy(
                            bpring[:, t, :], mo[:, :, 0:1].rearrange("p a b -> p (a b)")
                        )
                    nc.sync.dma_start(bp_d[:, ds(q, 1), :, :], bpring)

    return nc


def _prep_inputs(feats, transitions, T):
    transA = np.ascontiguousarray(
        transitions.reshape(NB, P, L).transpose(1, 0, 2)
    )                                                  # [p, k, j] = trans[128k+p, j]
    transT = np.ascontiguousarray(
        transitions.T.reshape(NB, P, L).transpose(1, 0, 2)
    )                                                  # [p, k, i] = trans[i, 128k+p]
    featsT4 = np.ascontiguousarray(
        feats.reshape(T, NB, P).transpose(2, 0, 1)
    )                                                  # [p, u, k] = feats[u, 128k+p]
    return {"transA": transA, "transT": transT, "featsT4": featsT4}


def _run_device(feats, transitions, T, trace=False):
    from concourse import bass_utils

    key = T
    if key not in _cache:
        _cache[key] = _build(T)
    nc = _cache[key]

    in_map = _prep_inputs(feats, transitions, T)
    res = bass_utils.run_bass_kernel_spmd(
        nc, [in_map] * NCORES, core_ids=list(range(NCORES)), trace=trace,
    )
    return res


def _assemble(res, T):
    pc_steps = T // NCORES
    final_row = np.asarray(res.results[0]["final_row"]).reshape(L)
    # bp per core: [P, ncq, CH, NB] uint16
    bps = [np.asarray(res.results[c]["bp"]) for c in range(NCORES)]

    last_tag = int(np.argmax(final_row))
    score = np.float32(final_row[last_tag])

    path = np.empty(T, dtype=np.int32)
    path[T - 1] = last_tag
    tag = last_tag
    for v in range(T - 2, -1, -1):
        c, r = divmod(v, pc_steps)
        q, t = divmod(r, CH)
        tag = int(bps[c][tag % P, q, t, tag // P])
        path[v] = tag
    return score, path


def kernel(feats, transitions):
    feats = np.ascontiguousarray(feats, dtype=np.float32)
    transitions = np.ascontiguousarray(transitions, dtype=np.float32)
    T = feats.shape[0]
    res = _run_device(feats, transitions, T)
    score, path = _assemble(res, T)
    return score, path


if __name__ == "__main__":
    rng = np.random.default_rng(0)
    T = int(os.environ.get("VITERBI_T", "1024"))
    feats = rng.standard_normal((T, L), dtype=np.float32)
    trans = rng.standard_normal((L, L), dtype=np.float32)
    score, path = kernel(feats, trans)
    print("score", score, "path head", path[:8])


# revision 18
# speedup vs baseline: 75.0554x; 1.3040x over previous
"""Viterbi decode (CRF) kernel for Trainium2.

Problem: feats [T=8192, L=512] emissions, transitions [L, L] (trans[i, j] =
score of i -> j). Returns (viterbi_score [], viterbi_path [T] int32),
matching the jax reference (lax.scan forward max-plus + backtrack).

Device strategy (8 NeuronCores, single NEFF, SPMD):

Phase 1 (all cores redundantly -- the scan is inherently serial):
  Layout A: source labels i on partitions (4 blocks of 128), dest labels j
  on the free dim. State s4p[p, k] = scores[128k + p] (per-partition
  scalars). Per step:
    - DVE: x = (trans_b1 + s_1); x = (trans_b0 + s_0) max x;
           x = (trans_b2 + s_2) max x; M = (trans_b3 + s_3) max x
      (tensor_scalar + 3x scalar_tensor_tensor, fused add+max tree)
    - PE: 4x transpose of M's 128-col blocks into one PSUM bank
    - DVE: one tensor_reduce(max) over [128, 4, 128] -> a4 [128, 4]
      (a4[p, b] = max_i(s[i] + trans[i, 128b + p]), pre-emission)
    - DVE: s4p = a4 + e4 (emission, partition-form)
    - off-chain: PE transpose s4p -> [4, 128] PSUM -> DMA as score row u
      to internal DRAM; a4 ring-buffered and flushed to internal DRAM.

Phase 2 (time-sharded across cores by partition_id): recompute m rows in
  layout B (dest j on partitions via transposed transitions) from stored
  score rows (K=1 fp32 matmul broadcast, bitwise-exact), take stored a4 as
  the exact max, and max_index -> first argmax index == jnp.argmax ties.

Host: trivial O(T) backtrack over the returned backpointer array.
"""

import os
import numpy as np

T_FULL = 8192
L = 512
P = 128
NB = 4          # label blocks of 128
CH = 32         # steps per hardware-loop chunk
NCORES = 8
NEG = -3.0e38

_cache = {}


def _wrap_birfix(nc):
    """Split multi-wait instructions into single-wait NoOp chains.

    This walrus build's CTRL encoder accepts only one semaphore wait per
    instruction; the Tile kernel-tail drain can carry several. Splitting is
    semantically identical (all waits still precede the instruction).
    Scoped to this Bass instance only.
    """
    import json as _json
    import types as _types
    import concourse.bass as _bass

    orig = _bass.Bass.to_json_bytes
    counter = [0]

    def to_json_bytes(self):
        bir = _json.loads(orig(self))
        changed = False
        for fn in bir.get("functions", []):
            for blk in fn.get("blocks", []):
                out = []
                for ins in blk.get("instructions", []):
                    si = ins.get("sync_info")
                    waits = (si or {}).get("on_wait") or []
                    if len(waits) > 1:
                        changed = True
                        for w in waits[:-1]:
                            counter[0] += 1
                            out.append({
                                "engine": ins["engine"],
                                "ins": [],
                                "name": f"IWSPLIT-{counter[0]}",
                                "opcode": "NoOp",
                                "outs": [],
                                "sync_info": {"on_update": [], "on_wait": [w]},
                            })
                        si["on_wait"] = waits[-1:]
                    out.append(ins)
                blk["instructions"] = out
        return _json.dumps(bir).encode() if changed else orig(self)

    nc.to_json_bytes = _types.MethodType(to_json_bytes, nc)
    return nc


def _build(T):
    import concourse.bass as bass
    import concourse.mybir as mybir
    from concourse.bass import ds
    from concourse.tile import TileContext
    from concourse.masks import make_identity

    f32 = mybir.dt.float32
    u16 = mybir.dt.uint16
    ALU = mybir.AluOpType

    steps = T - 1                 # scan steps u = 1..T-1
    nch1 = steps // CH            # full phase-1 chunks
    tail1 = steps - nch1 * CH     # python-unrolled tail steps
    pc_steps = T // NCORES        # phase-2 steps per core
    ncq = pc_steps // CH          # phase-2 chunks per core

    nc = _wrap_birfix(bass.Bass())

    transA_d = nc.dram_tensor("transA", [P, NB, L], f32, kind="ExternalInput")
    transT_d = nc.dram_tensor("transT", [P, NB, L], f32, kind="ExternalInput")
    featsT4_d = nc.dram_tensor("featsT4", [P, T, NB], f32, kind="ExternalInput")

    rows_d = nc.dram_tensor("rows_i", [NB, T, P], f32, kind="Internal")
    a4_d = nc.dram_tensor("a4_i", [P, T + CH, NB], f32, kind="Internal")

    frow_d = nc.dram_tensor("final_row", [1, L], f32, kind="ExternalOutput")
    bp_d = nc.dram_tensor("bp", [P, ncq, CH, NB], u16, kind="ExternalOutput")

    with TileContext(nc) as tc:
        with (
            tc.tile_pool(name="consts", bufs=1) as consts,
            tc.tile_pool(name="state", bufs=1) as statep,
        ):
            ident = consts.tile([P, P], f32, tag="ident")
            make_identity(nc, ident)
            ones_row = consts.tile([1, P], f32, tag="ones_row")
            nc.vector.memset(ones_row, 1.0)
            s4p = statep.tile([P, NB], f32, tag="s4p")

            # ---------------- Phase 1: serial forward scan ----------------
            with (
                tc.tile_pool(name="p1sbuf", bufs=3) as pool,
                tc.tile_pool(name="p1ring", bufs=2) as ringp,
                tc.tile_pool(name="p1psum", bufs=2, space="PSUM") as psum,
                tc.tile_pool(name="p1psrow", bufs=4, space="PSUM") as psrow,
            ):
                transA = consts.tile([P, NB, L], f32, tag="transA")
                nc.sync.dma_start(transA, transA_d[:, :, :])

                # init: s4p = e4(0) = feats[0] in partition form
                e40 = pool.tile([P, 1, NB], f32, tag="e4chunk")
                nc.sync.dma_start(e40, featsT4_d[:, 0:1, :])
                nc.vector.tensor_copy(s4p, e40[:, 0, :])
                # scores_rows[0] = feats[0]
                srow0 = psrow.tile([NB, P], f32, tag="srow")
                nc.tensor.transpose(srow0, s4p, ident)
                srow0_sb = pool.tile([NB, P], f32, tag="srow0sb")
                nc.scalar.copy(srow0_sb, srow0)
                nc.sync.dma_start(rows_d[:, 0:1, :], srow0_sb)

                def step(u_expr, e4_ap, a4_slot_ap, row_slot_ap):
                    """One scan step; u_expr only used via the passed APs."""
                    x = pool.tile([P, L], f32, tag="x")
                    nc.vector.tensor_scalar(
                        x, transA[:, 1, :], s4p[:, 1:2], None, op0=ALU.add
                    )
                    for b in (0, 2):
                        nc.vector.scalar_tensor_tensor(
                            out=x, in0=transA[:, b, :], scalar=s4p[:, b:b + 1],
                            in1=x, op0=ALU.add, op1=ALU.max,
                        )
                    M = pool.tile([P, L], f32, tag="M")
                    MT = psum.tile([P, NB, P], f32, tag="MT")
                    for b in range(NB):
                        cs = slice(b * P, (b + 1) * P)
                        nc.vector.scalar_tensor_tensor(
                            out=M[:, cs], in0=transA[:, 3, cs], scalar=s4p[:, 3:4],
                            in1=x[:, cs], op0=ALU.add, op1=ALU.max,
                        )
                        nc.tensor.transpose(MT[:, b, :], M[:, cs], ident)
                    a4 = pool.tile([P, NB], f32, tag="a4")
                    nc.vector.tensor_reduce(a4, MT, axis=mybir.AxisListType.X, op=ALU.max)
                    # stash pre-emission maxes (exact) for phase 2
                    nc.scalar.copy(a4_slot_ap, a4)
                    # s4p_new = a4 + emission(u)
                    nc.vector.tensor_tensor(s4p, a4, e4_ap, op=ALU.add)
                    # row store: transpose s4p -> [4, 128] -> SBUF ring slot
                    srow = psrow.tile([NB, P], f32, tag="srow")
                    nc.tensor.transpose(srow, s4p, ident)
                    nc.scalar.copy(row_slot_ap, srow)

                with tc.For_i(0, nch1, 1) as ci:
                    base = ci * CH + 1   # first u of this chunk
                    e4c = pool.tile([P, CH, NB], f32, tag="e4chunk")
                    nc.sync.dma_start(e4c, featsT4_d[:, ds(base, CH), :])
                    a4buf = ringp.tile([P, CH, NB], f32, tag="a4buf")
                    rowbuf = ringp.tile([NB, CH, P], f32, tag="rowbuf")
                    for t in range(CH):
                        step(
                            None,
                            e4c[:, t, :],
                            a4buf[:, t, :],
                            rowbuf[:, t, :],
                        )
                    nc.sync.dma_start(a4_d[:, ds(base, CH), :], a4buf)
                    nc.sync.dma_start(rows_d[:, ds(base, CH), :], rowbuf)

                if tail1:
                    base = nch1 * CH + 1
                    e4c = pool.tile([P, CH, NB], f32, tag="e4chunk")
                    nc.sync.dma_start(e4c[:, 0:tail1, :], featsT4_d[:, base:base + tail1, :])
                    a4buf = ringp.tile([P, CH, NB], f32, tag="a4buf")
                    rowbuf = ringp.tile([NB, CH, P], f32, tag="rowbuf")
                    for t in range(tail1):
                        step(
                            None,
                            e4c[:, t, :],
                            a4buf[:, t, :],
                            rowbuf[:, t, :],
                        )
                    nc.sync.dma_start(
                        a4_d[:, base:base + tail1, :], a4buf[:, 0:tail1, :]
                    )
                    nc.sync.dma_start(
                        rows_d[:, base:base + tail1, :], rowbuf[:, 0:tail1, :]
                    )

            tc.strict_bb_all_engine_barrier()

            # ---------------- Phase 2: backpointers, time-sharded ----------------
            with (
                tc.tile_pool(name="p2sbuf", bufs=3) as pool,
                tc.tile_pool(name="p2m", bufs=3) as mpool,
                tc.tile_pool(name="p2psum", bufs=2, space="PSUM") as psum,
            ):
                transT = consts.tile([P, NB, L], f32, tag="transT")
                nc.sync.dma_start(transT, transT_d[:, :, :])

                # final scores row for the host (after the barrier so the
                # phase-1 row writes are complete)
                frow_sb = pool.tile([1, L], f32, tag="frowsb")
                nc.sync.dma_start(frow_sb, rows_d[:, T - 1:T, :])
                nc.sync.dma_start(frow_d[:, :], frow_sb)
                inmax = consts.tile([P, NB, 8], f32, tag="inmax")
                nc.vector.memset(inmax, 3.0e38)

                pid = nc.partition_id()

                with tc.For_i(0, ncq, 1) as q:
                    vbase = pid * pc_steps + q * CH   # v = u - 1 for 1st step
                    a4c = pool.tile([P, CH, NB], f32, tag="a4c")
                    nc.sync.dma_start(a4c, a4_d[:, ds(vbase + 1, CH), :])
                    rows_c = pool.tile([NB, CH, P], f32, tag="rows_c")
                    nc.sync.dma_start(rows_c, rows_d[:, ds(vbase, CH), :])
                    bpring = pool.tile([P, CH, NB], u16, tag="bpring")
                    for t in range(CH):
                        rowt = pool.tile([1, L], f32, tag="rowt")
                        nc.sync.dma_start(rowt, rows_c[:, t, :])
                        rep = psum.tile([P, L], f32, tag="rep")
                        nc.tensor.matmul(
                            rep, ones_row, rowt, start=True, stop=True
                        )
                        rep_sb = mpool.tile([P, L], f32, tag="rep_sb")
                        nc.scalar.copy(rep_sb, rep)
                        # exact max values into slot 0 of the 8-wide in_max rows
                        nc.vector.tensor_copy(inmax[:, :, 0:1], a4c[:, t:t + 1, :].rearrange("p a b -> p b a"))
                        mo = mpool.tile([P, NB, 8], u16, tag="midxo")
                        ms = []
                        for k in range(NB):
                            m = mpool.tile([P, L], f32, tag="m")
                            eng = nc.gpsimd if k in (0, 1, 2) else nc.vector
                            eng.tensor_tensor(m, transT[:, k, :], rep_sb, op=ALU.add)
                            ms.append(m)
                        for k in range(NB):
                            nc.vector.max_index(mo[:, k, :], inmax[:, k, :], ms[k])
                        nc.vector.tensor_copy(
                            bpring[:, t, :], mo[:, :, 0:1].rearrange("p a b -> p (a b)")
                        )
                    nc.sync.dma_start(bp_d[:, ds(q, 1), :, :], bpring)

    return nc


def _prep_inputs(feats, transitions, T):
    transA = np.ascontiguousarray(
        transitions.reshape(NB, P, L).transpose(1, 0, 2)
    )                                                  # [p, k, j] = trans[128k+p, j]
    transT = np.ascontiguousarray(
        transitions.T.reshape(NB, P, L).transpose(1, 0, 2)
    )                                                  # [p, k, i] = trans[i, 128k+p]
    featsT4 = np.ascontiguousarray(
        feats.reshape(T, NB, P).transpose(2, 0, 1)
    )                                                  # [p, u, k] = feats[u, 128k+p]
    return {"transA": transA, "transT": transT, "featsT4": featsT4}


def _run_device(feats, transitions, T, trace=False):
    from concourse import bass_utils

    key = T
    if key not in _cache:
        _cache[key] = _build(T)
    nc = _cache[key]

    in_map = _prep_inputs(feats, transitions, T)
    res = bass_utils.run_bass_kernel_spmd(
        nc, [in_map] * NCORES, core_ids=list(range(NCORES)), trace=trace,
    )
    return res


def _assemble(res, T):
    pc_steps = T // NCORES
    final_row = np.asarray(res.results[0]["final_row"]).reshape(L)
    # bp per core: [P, ncq, CH, NB] uint16
    bps = [np.asarray(res.results[c]["bp"]) for c in range(NCORES)]

    last_tag = int(np.argmax(final_row))
    score = np.float32(final_row[last_tag])

    path = np.empty(T, dtype=np.int32)
    path[T - 1] = last_tag
    tag = last_tag
    for v in range(T - 2, -1, -1):
        c, r = divmod(v, pc_steps)
        q, t = divmod(r, CH)
        tag = int(bps[c][tag % P, q, t, tag // P])
        path[v] = tag
    return score, path


def kernel(feats, transitions):
    feats = np.ascontiguousarray(feats, dtype=np.float32)
    transitions = np.ascontiguousarray(transitions, dtype=np.float32)
    T = feats.shape[0]
    res = _run_device(feats, transitions, T)
    score, path = _assemble(res, T)
    return score, path


if __name__ == "__main__":
    rng = np.random.default_rng(0)
    T = int(os.environ.get("VITERBI_T", "1024"))
    feats = rng.standard_normal((T, L), dtype=np.float32)
    trans = rng.standard_normal((L, L), dtype=np.float32)
    score, path = kernel(feats, trans)
    print("score", score, "path head", path[:8])
